# revision 55
# baseline (speedup 1.0000x reference)
"""CBAM channel attention kernel for Trainium2 (8 NeuronCores, batch-parallel).

x: [32, 768, 56, 56] f32 on host, cast to bf16 for the device pass (38.5 MB
HBM traffic per core round-trip, ~93 us fabric floor at 435 GB/s).  Each core
handles 4 samples; channel-chunk pairs [128, 2, 3136] stay resident in SBUF
between pooling and scaling so HBM traffic is exactly 1 read + 1 write of x.

The schedule is built around ScalarE (ACT), the throughput pacer: ~22
sum-pool Copies (2.9us each) + 4 gate chains ~= 80us of dense ACT stream.
Everything else is phased (tile_wait_until as a sim-scheduler priority key)
to keep both ACT and DVE dense:
 1. Sample 0's reads go per-chunk (first chunk split in half) so the first
    ACT sum starts ~10.4us; sample 0's chunk-5 sum runs on DVE (add-tree +
    CACHE_REDUCE add) where DVE has fill-phase slack.
 2. Sum-pools for chunks 0-3 of sample b+1 are phased into sample b's gate
    window as ACT fillers; ERF sits after the 2nd filler and the serial
    12-op matmul2 chain hides under the following filler before SIGMOID.
 3. TREES-FIRST on DVE: ALL of sample b+1's max-trees are queued before
    sample b's gate multiplies.  sig_b is ready long before the trees end,
    so DVE never stalls, and the CACHE_REDUCEs are ready when ERF needs
    matmul1 (this removed a measured 8us ACT stall before the last ERF).
 4. The whole hh/hsum gelu algebra runs on ACT via the activation affine
    (func=Identity, scale=bias=u): ANY DVE op that waits on an ACT result
    mid-queue re-serializes the next sample's trees behind the gate chain.
 5. The last sample runs a per-chunk matmul2 -> sigmoid -> multiply ->
    write pipeline, writes spread across gpsimd/sync/scalar queues so the
    final dge_drain hides under HWDGE transfers.
 6. The 1/HW mean scale rides the sum-pool Copy's free affine; a dummy
    sigmoid pins the sigmoid table-set (holds erf+copy) so only one
    ACT_TABLE_LOAD happens.

Pooling: max on DVE as a depth-3 tensor_tensor max tree (bf16 2x pump) +
one segmented tensor_reduce per PAIR at 392 wide; sum on ScalarE (Copy +
accum_out, main output to a zero-stride sink).  MLP in transposed form on
TensorE with host-pretransposed f32 weights; exact gelu via Erf (0.5 folded
into w2t).  Reads ride the Sync HWDGE ring; steady-state writes ride SWDGE.

Measured dead ends (do not retry): GpSimd tensor_tensor offload (shared
SBUF port slows concurrent DVE pumped ops 2.7x); fp8 e4m3 tail writes
(correct at 1.35e-2 rel err but not faster); avg-pool as W1@x on TensorE
(PSUM one-bank-per-matmul forces 42 small MMs/sample).
"""

import ml_dtypes
import numpy as np

import concourse.bacc as bacc
import concourse.bass as bass
import concourse.mybir as mybir
import concourse.tile as tile
from concourse.bass_utils import run_bass_kernel_spmd

B = 32
C = 768
HW = 56 * 56    # 3136
HWH = HW // 2   # 1568
HWQ = HW // 4   # 784
HID = 48        # C // 16
NCORES = 8
B_LOC = B // NCORES  # 4
KC = C // 128        # 6 channel chunks
F32 = mybir.dt.float32
BF16 = mybir.dt.bfloat16
AF = mybir.ActivationFunctionType
ALU = mybir.AluOpType

_cache = {}


def _build_nc():
    nc = bacc.Bacc("TRN2", target_bir_lowering=False, debug=False)
    x_d = nc.declare_dram_parameter("x", [B_LOC * C, HW], BF16, isOutput=False)
    # host-pretransposed weights: w1t[p, k, h] = w1[h, k*128+p],
    # w2t[h, k, p] = 0.5 * w2[k*128+p, h]  (0.5 folds the gelu half)
    w1_d = nc.declare_dram_parameter("w1t", [128, KC * HID], F32, isOutput=False)
    w2_d = nc.declare_dram_parameter("w2t", [HID, KC * 128], F32, isOutput=False)
    out_d = nc.declare_dram_parameter("out", [B_LOC * C, HW], BF16,
                                      isOutput=True)

    with tile.TileContext(nc) as tc:
        with (
            tc.tile_pool(name="consts", bufs=1) as consts,
            tc.tile_pool(name="big", bufs=10) as bigpool,
            tc.tile_pool(name="ttree", bufs=2) as tpool,
            tc.tile_pool(name="pooled", bufs=3) as pooled_pool,
            tc.tile_pool(name="small", bufs=3) as small_pool,
            tc.tile_pool(name="psum", bufs=2, space="PSUM") as psum_pool,
        ):
            sink = consts.tile([128, 1], BF16)
            # write-only scratch for the CACHE_REDUCE output streams
            garbage = consts.tile([128, HWQ], BF16)
            # dummy sigmoid: pin the sigmoid table-set (contains erf + copy)
            # before the first sum-pool so only ONE ACT_TABLE_LOAD happens
            with tc.tile_wait_until(0.001):
                nc.scalar.activation(out=sink[:, 0:1], in_=sink[:, 0:1],
                                     func=AF.Sigmoid)

            all_ots = []
            all_pooled = []

            # ---- reads: greedy, Sync HWDGE ring.  Sample 0 goes per-chunk
            # so the first pool ops start as soon as ~0.8 MB lands. ----
            for b in range(B_LOC):
                ots = []
                for j in range(KC // 2):
                    ot = bigpool.tile([128, 2, HW], BF16, tag="o", bufs=10,
                                      name=f"ot{b}_{j}")
                    row = (b * KC + 2 * j) * 128
                    if b == 0:
                        for i in range(2):
                            if j == 0 and i == 0:
                                # very first chunk split in half so the first
                                # ACT sum starts ~1.5us earlier
                                nc.sync.dma_start(
                                    out=ot[:, 0, 0:HWH],
                                    in_=x_d[row : row + 128, 0:HWH],
                                )
                                nc.sync.dma_start(
                                    out=ot[:, 0, HWH:HW],
                                    in_=x_d[row : row + 128, HWH:HW],
                                )
                                continue
                            nc.sync.dma_start(
                                out=ot[:, i, :],
                                in_=x_d[row + 128 * i : row + 128 * (i + 1), :],
                            )
                    else:
                        nc.sync.dma_start(
                            out=ot,
                            in_=x_d[row : row + 256, :].rearrange(
                                "(k p) f -> p k f", p=128
                            ),
                        )
                    ots.append(ot)
                all_ots.append(ots)
                all_pooled.append(
                    pooled_pool.tile([128, KC + 1, 2], F32, name=f"pooled{b}")
                )
                if b == 0:
                    # weights ride the sync ring AFTER sample 0's chunks:
                    # they aren't needed until matmul1 (~25us in), and
                    # triggering them first would delay the first pool ops
                    w1T = consts.tile([128, KC, HID], F32)
                    nc.sync.dma_start(
                        out=w1T, in_=w1_d.rearrange("p (k h) -> p k h", k=KC)
                    )
                    w2T = consts.tile([HID, KC, 128], F32)
                    nc.sync.dma_start(
                        out=w2T, in_=w2_d.rearrange("h (k p) -> h k p", k=KC)
                    )

            def act_sum(b, k, ph):
                with tc.tile_wait_until(ph):
                    nc.scalar.activation(
                        out=sink[:, 0:1].to_broadcast([128, HW]),
                        in_=all_ots[b][k // 2][:, k % 2, :],
                        func=AF.Copy,
                        scale=1.0 / HW,
                        accum_out=all_pooled[b][:, k, 0:1],
                    )

            def dve_sum(b, k, ph):
                # add-tree + CACHE_REDUCE(add): sum-pool on DVE for the
                # fill phase where DVE has slack and ACT is the backlog.
                # bf16 intermediates round ~2^-9 per level; the CR
                # accumulates in f32, well inside the 2e-2 error budget.
                with tc.tile_wait_until(ph):
                    src = all_ots[b][k // 2][:, k % 2, :]
                    s1 = tpool.tile([128, HWH], BF16, tag="s1", bufs=1,
                                    name=f"s1_{b}_{k}")
                    nc.vector.tensor_tensor(
                        out=s1, in0=src[0:128, 0:HWH], in1=src[0:128, HWH:HW],
                        op=ALU.add,
                    )
                    s2 = tpool.tile([128, HWQ], BF16, tag="s2", bufs=1,
                                    name=f"s2_{b}_{k}")
                    nc.vector.tensor_tensor(
                        out=s2, in0=s1[:, 0:HWQ], in1=s1[:, HWQ:HWH],
                        op=ALU.add,
                    )
                    nc.vector.tensor_scalar(
                        out=garbage[:, 0:HWQ],
                        in0=s2,
                        scalar1=1.0 / HW,
                        scalar2=None,
                        op0=ALU.mult,
                        op1=ALU.add,
                        accum_out=all_pooled[b][:, k, 0:1],
                    )

            HWE = HWQ // 2  # 392

            def max_tree(b, j, ph):
                with tc.tile_wait_until(ph):
                    ot = all_ots[b][j]
                    t1 = tpool.tile([128, 2, HWH], BF16, tag="t1", bufs=1,
                                    name=f"t1_{b}_{j}")
                    nc.vector.tensor_tensor(
                        out=t1, in0=ot[:, :, 0:HWH], in1=ot[:, :, HWH:HW],
                        op=ALU.max,
                    )
                    t2 = tpool.tile([128, 2, HWQ], BF16, tag="t2", bufs=1,
                                    name=f"t2_{b}_{j}")
                    nc.vector.tensor_tensor(
                        out=t2, in0=t1[:, :, 0:HWQ], in1=t1[:, :, HWQ:HWH],
                        op=ALU.max,
                    )
                    t3 = tpool.tile([128, 2, HWE], BF16, tag="t3", bufs=1,
                                    name=f"t3_{b}_{j}")
                    nc.vector.tensor_tensor(
                        out=t3, in0=t2[:, :, 0:HWE], in1=t2[:, :, HWE:HWQ],
                        op=ALU.max,
                    )
                    # one segmented reduce finishes BOTH chunks of the pair
                    nc.vector.tensor_reduce(
                        out=all_pooled[b][:, 2 * j : 2 * j + 2, 1],
                        in_=t3,
                        axis=mybir.AxisListType.X,
                        op=ALU.max,
                    )

            # ---- per-sample emission ----
            for b in range(B_LOC):
                ots = all_ots[b]
                pooled = all_pooled[b]
                last = b == B_LOC - 1

                # ACT sum-pools.  Window 0: chunks 0-4 on ACT in read-arrival
                # order, chunk 5 on DVE after the trees.  Steady state:
                # chunks 0-3 of sample b are fillers in window b-1's gate
                # chain; chunks 4,5 stay in window b.
                if b == 0:
                    # chunk 0 as two half-sums (halves land first); the
                    # second half accumulates into the spare pooled slot KC,
                    # folded back by an extra n=1 matmul1 term
                    with tc.tile_wait_until(0.005):
                        nc.scalar.activation(
                            out=sink[:, 0:1].to_broadcast([128, HWH]),
                            in_=ots[0][:, 0, 0:HWH],
                            func=AF.Copy, scale=1.0 / HW,
                            accum_out=pooled[:, 0, 0:1],
                        )
                    with tc.tile_wait_until(0.008):
                        nc.scalar.activation(
                            out=sink[:, 0:1].to_broadcast([128, HWH]),
                            in_=ots[0][:, 0, HWH:HW],
                            func=AF.Copy, scale=1.0 / HW,
                            accum_out=pooled[:, KC, 0:1],
                        )
                    for k in range(1, 5):
                        act_sum(0, k, 0.01 + 0.01 * k)
                else:
                    # NOTE: offloading sums to GpSimd tensor_tensor does NOT
                    # work: GpSimd shares an SBUF port with DVE, and running
                    # them concurrently was measured to slow DVE's pumped
                    # tensor ops 2.7x (MAX 1.7us -> 4.65us).
                    # (Moving c0 fillers to DVE add-trees was also tried:
                    # ERF_3 didn't move -- those windows aren't ACT-reach
                    # bound -- and it opened a new fabric gap.)
                    for k in range(4):
                        act_sum(b, k, (b - 1) + (0.90, 0.92, 0.94, 0.95)[k])
                    for k in range(4, KC):
                        act_sum(b, k, b + 0.10 + 0.02 * (k - 4))

                # DVE max-trees in read-arrival order.  ALL of sample b's
                # trees run BEFORE sample b-1's multiplies (trees-first):
                # sig_{b-1} is ready long before the trees finish, so DVE
                # never stalls, and the CACHE_REDUCEs are ready early for
                # matmul1/erf.
                if b == 0:
                    for j in range(KC // 2):
                        max_tree(0, j, 0.06 + 0.005 * j)
                    dve_sum(0, 5, 0.075)
                else:
                    for j in range(KC // 2):
                        max_tree(b, j, b + 0.10 + 0.01 * j)

                # matmul1: hT [48, 2] = sum_k w1T_k.T @ pooledT_k
                hps = psum_pool.tile([HID, 2], F32, tag="hps", name=f"hps{b}")
                for k in range(KC):
                    with tc.tile_wait_until(b + 0.30 + 0.01 * k):
                        nc.tensor.matmul(
                            hps,
                            w1T[:, k, :],
                            pooled[:, k, :],
                            start=(k == 0),
                            stop=(k == KC - 1),
                        )
                        if b == 0 and k == 0:
                            # fold the split first chunk's second half-sum
                            nc.tensor.matmul(
                                hps[:, 0:1],
                                w1T[:, 0, :],
                                pooled[:, KC, 0:1],
                                start=False,
                                stop=False,
                            )

                # gate chain: erf -> hh/hsum -> matmul2 -> sigmoid, ALL on
                # ACT.  Keeping hh/hsum off DVE matters: a DVE op waiting on
                # erf mid-queue serializes the next sample's max-trees behind
                # ACT's chain (measured 8us ACT idle before the last ERF).
                # hh_j = (e_j + 1) * u_j via the activation affine
                # (out = func(in*scale + bias)) with scale = bias = u_j.
                # sample 2 only: erf/sig pulled two filler slots earlier
                # (their CACHE_REDUCE deps are ready ~8us before ACT's
                # natural reach) so M_2 becomes trees-end-gated instead of
                # sigmoid-gated, shrinking the 5.5us tail fabric gap
                if b == 0:
                    erf_ph = 0.91
                elif b == 2:
                    erf_ph = b + 0.905
                else:
                    erf_ph = b + 0.945
                with tc.tile_wait_until(erf_ph - 0.001):
                    u_sb = small_pool.tile([HID, 2], F32, tag="u",
                                           name=f"u{b}")
                    nc.scalar.activation(out=u_sb, in_=hps, func=AF.Copy)
                with tc.tile_wait_until(erf_ph):
                    e_sb = small_pool.tile([HID, 2], F32, tag="e",
                                           name=f"e{b}")
                    nc.scalar.activation(
                        out=e_sb, in_=hps, func=AF.Erf, scale=0.7071067811865476
                    )
                with tc.tile_wait_until(erf_ph + 0.002):
                    hh = small_pool.tile([HID, 2], F32, tag="hh", name=f"hh{b}")
                    for jj in range(2):
                        nc.scalar.activation(
                            out=hh[:, jj : jj + 1], in_=e_sb[:, jj : jj + 1],
                            func=AF.Identity,
                            scale=u_sb[:, jj : jj + 1],
                            bias=u_sb[:, jj : jj + 1],
                        )
                    hsum = small_pool.tile([HID, 1], F32, tag="hsum",
                                           name=f"hsum{b}")
                    nc.scalar.activation(
                        out=hsum, in_=hh[:, 0:1], func=AF.Identity,
                        bias=hh[:, 1:2],
                    )
                mlp = psum_pool.tile([128, KC], F32, tag="mlp", name=f"mlp{b}")
                gate = small_pool.tile([128, KC], F32, tag="gate",
                                       name=f"gate{b}")
                if b == 0:
                    sig_ph = 0.93
                elif b == 2:
                    sig_ph = b + 0.926
                else:
                    sig_ph = b + 0.965
                if not last:
                    for k in range(KC):
                        with tc.tile_wait_until(erf_ph + 0.004 + 0.001 * k):
                            nc.tensor.matmul(
                                mlp[:, k : k + 1],
                                w2T[:, k, :],
                                hsum,
                                start=True,
                                stop=True,
                            )
                    with tc.tile_wait_until(sig_ph):
                        nc.scalar.activation(out=gate, in_=mlp, func=AF.Sigmoid)

                    # multiplies + writes: window b+1, after ALL of sample
                    # b+1's trees on DVE (trees-first ordering)
                    for j in range(KC // 2):
                        with tc.tile_wait_until(b + 1.16 + 0.01 * j):
                            ot = ots[j]
                            row = (b * KC + 2 * j) * 128
                            wt = bigpool.tile([128, 2, HW], BF16, tag="w",
                                              bufs=4, name=f"wt{b}_{j}")
                            for i in range(2):
                                k = 2 * j + i
                                nc.vector.tensor_scalar_mul(
                                    wt[:, i, :], ot[:, i, :], gate[:, k : k + 1]
                                )
                            out_ap = out_d[row : row + 256, :].rearrange(
                                "(k p) f -> p k f", p=128
                            )
                            nc.gpsimd.dma_start(out=out_ap, in_=wt)
                else:
                    # last sample: per-chunk matmul2 -> sigmoid -> mult ->
                    # write pipeline; writes spread across gpsimd/sync/
                    # scalar queues.  Write tiles reuse the steady-state
                    # "w" pair tag, half each.  (fp8 tail writes were tried:
                    # correct at 1.35e-2 rel err but NOT faster -- the DVE
                    # fp8-out mult loses its pump and SWDGE-cast serializes.)
                    wts = [
                        bigpool.tile([128, 2, HW], BF16, tag="w", bufs=4,
                                     name=f"wtl{j}")
                        for j in range(KC // 2)
                    ]
                    for k in range(KC):
                        with tc.tile_wait_until(b + 0.95 + 0.002 * k):
                            nc.tensor.matmul(
                                mlp[:, k : k + 1],
                                w2T[:, k, :],
                                hsum,
                                start=True,
                                stop=True,
                            )
                            nc.scalar.activation(
                                out=gate[:, k : k + 1], in_=mlp[:, k : k + 1],
                                func=AF.Sigmoid,
                            )
                        with tc.tile_wait_until(b + 0.96 + 0.002 * k):
                            ot = ots[k // 2]
                            row = (b * KC + k) * 128
                            wt = wts[k // 2][:, k % 2, :]
                            nc.vector.tensor_scalar_mul(
                                wt, ot[:, k % 2, :], gate[:, k : k + 1]
                            )
                            eng = (nc.gpsimd, nc.gpsimd, nc.sync, nc.sync,
                                   nc.scalar, nc.scalar)[k]
                            eng.dma_start(
                                out=out_d[row : row + 128, :], in_=wt
                            )
    nc.finalize()
    return nc


def kernel(x, w1, w2, _trace=False):
    if "nc" not in _cache:
        _cache["nc"] = _build_nc()
    nc = _cache["nc"]

    x = np.asarray(x).reshape(B, C, HW)
    w1t = np.ascontiguousarray(
        np.asarray(w1, np.float32).reshape(HID, KC, 128).transpose(2, 1, 0)
        .reshape(128, KC * HID)
    )
    w2t = np.ascontiguousarray(
        (0.5 * np.asarray(w2, np.float32)).reshape(KC, 128, HID)
        .transpose(2, 0, 1).reshape(HID, KC * 128)
    )
    in_maps = [
        {
            "x": np.ascontiguousarray(
                x[i * B_LOC : (i + 1) * B_LOC].reshape(B_LOC * C, HW)
            ).astype(ml_dtypes.bfloat16),
            "w1t": w1t,
            "w2t": w2t,
        }
        for i in range(NCORES)
    ]
    res = run_bass_kernel_spmd(nc, in_maps, core_ids=list(range(NCORES)),
                               trace=_trace)
    out = np.concatenate(
        [
            r["out"].astype(np.float32).reshape(B_LOC, C, 56, 56)
            for r in res.results
        ],
        axis=0,
    )
    if _trace:
        _cache["last_results"] = res
    return out


# revision 57
# speedup vs baseline: 1.0390x; 1.0390x over previous
"""CBAM channel attention kernel for Trainium2 (8 NeuronCores, batch-parallel).

x: [32, 768, 56, 56] f32 on host, cast to bf16 for the device pass (38.5 MB
HBM traffic per core round-trip, ~93 us fabric floor at 435 GB/s).  Each core
handles 4 samples; channel-chunk pairs [128, 2, 3136] stay resident in SBUF
between pooling and scaling so HBM traffic is exactly 1 read + 1 write of x.

The schedule is built around ScalarE (ACT), the throughput pacer: ~22
sum-pool Copies (2.9us each) + 4 gate chains ~= 80us of dense ACT stream.
Everything else is phased (tile_wait_until as a sim-scheduler priority key)
to keep both ACT and DVE dense:
 1. Sample 0's reads go per-chunk (first chunk split in half) so the first
    ACT sum starts ~10.4us; sample 0's chunk-5 sum runs on DVE (add-tree +
    CACHE_REDUCE add) where DVE has fill-phase slack.
 2. Sum-pools for chunks 0-3 of sample b+1 are phased into sample b's gate
    window as ACT fillers; ERF sits after the 2nd filler and the serial
    12-op matmul2 chain hides under the following filler before SIGMOID.
 3. TREES-FIRST on DVE: ALL of sample b+1's max-trees are queued before
    sample b's gate multiplies.  sig_b is ready long before the trees end,
    so DVE never stalls, and the CACHE_REDUCEs are ready when ERF needs
    matmul1 (this removed a measured 8us ACT stall before the last ERF).
 4. The whole hh/hsum gelu algebra runs on ACT via the activation affine
    (func=Identity, scale=bias=u): ANY DVE op that waits on an ACT result
    mid-queue re-serializes the next sample's trees behind the gate chain.
 5. The last sample runs a per-chunk matmul2 -> sigmoid -> multiply ->
    write pipeline, writes spread across gpsimd/sync/scalar queues so the
    final dge_drain hides under HWDGE transfers.
 6. The 1/HW mean scale rides the sum-pool Copy's free affine; a dummy
    sigmoid pins the sigmoid table-set (holds erf+copy) so only one
    ACT_TABLE_LOAD happens.

Pooling: max on DVE as a depth-3 tensor_tensor max tree (bf16 2x pump) +
one segmented tensor_reduce per PAIR at 392 wide; sum on ScalarE (Copy +
accum_out, main output to a zero-stride sink).  MLP in transposed form on
TensorE with host-pretransposed f32 weights; exact gelu via Erf (0.5 folded
into w2t).  Reads ride the Sync HWDGE ring; steady-state writes ride SWDGE.

Measured dead ends (do not retry): GpSimd tensor_tensor offload (shared
SBUF port slows concurrent DVE pumped ops 2.7x); fp8 e4m3 tail writes
(correct at 1.35e-2 rel err but not faster); avg-pool as W1@x on TensorE
(PSUM one-bank-per-matmul forces 42 small MMs/sample).
"""

import ml_dtypes
import numpy as np

import concourse.bacc as bacc
import concourse.bass as bass
import concourse.mybir as mybir
import concourse.tile as tile
from concourse.bass_utils import run_bass_kernel_spmd

B = 32
C = 768
HW = 56 * 56    # 3136
HWH = HW // 2   # 1568
HWQ = HW // 4   # 784
HID = 48        # C // 16
NCORES = 8
B_LOC = B // NCORES  # 4
KC = C // 128        # 6 channel chunks
F32 = mybir.dt.float32
BF16 = mybir.dt.bfloat16
AF = mybir.ActivationFunctionType
ALU = mybir.AluOpType

_cache = {}


def _build_nc():
    nc = bacc.Bacc("TRN2", target_bir_lowering=False, debug=False)
    x_d = nc.declare_dram_parameter("x", [B_LOC * C, HW], BF16, isOutput=False)
    # host-pretransposed weights: w1t[p, k, h] = w1[h, k*128+p],
    # w2t[h, k, p] = 0.5 * w2[k*128+p, h]  (0.5 folds the gelu half)
    w1_d = nc.declare_dram_parameter("w1t", [128, KC * HID], F32, isOutput=False)
    w2_d = nc.declare_dram_parameter("w2t", [HID, KC * 128], F32, isOutput=False)
    out_d = nc.declare_dram_parameter("out", [B_LOC * C, HW], BF16,
                                      isOutput=True)

    with tile.TileContext(nc) as tc:
        with (
            tc.tile_pool(name="consts", bufs=1) as consts,
            tc.tile_pool(name="big", bufs=10) as bigpool,
            tc.tile_pool(name="ttree", bufs=2) as tpool,
            tc.tile_pool(name="pooled", bufs=3) as pooled_pool,
            tc.tile_pool(name="small", bufs=3) as small_pool,
            tc.tile_pool(name="psum", bufs=2, space="PSUM") as psum_pool,
        ):
            sink = consts.tile([128, 1], BF16)
            # write-only scratch for the CACHE_REDUCE output streams
            garbage = consts.tile([128, HWQ], BF16)
            # dummy sigmoid: pin the sigmoid table-set (contains erf + copy)
            # before the first sum-pool so only ONE ACT_TABLE_LOAD happens
            with tc.tile_wait_until(0.001):
                nc.scalar.activation(out=sink[:, 0:1], in_=sink[:, 0:1],
                                     func=AF.Sigmoid)

            all_ots = []
            all_pooled = []

            # ---- reads: greedy, Sync HWDGE ring.  Sample 0 goes per-chunk
            # so the first pool ops start as soon as ~0.8 MB lands. ----
            for b in range(B_LOC):
                ots = []
                for j in range(KC // 2):
                    ot = bigpool.tile([128, 2, HW], BF16, tag="o", bufs=10,
                                      name=f"ot{b}_{j}")
                    row = (b * KC + 2 * j) * 128
                    if b == 0:
                        for i in range(2):
                            if j == 0 and i == 0:
                                # very first chunk split in half so the first
                                # ACT sum starts ~1.5us earlier
                                nc.sync.dma_start(
                                    out=ot[:, 0, 0:HWH],
                                    in_=x_d[row : row + 128, 0:HWH],
                                )
                                nc.sync.dma_start(
                                    out=ot[:, 0, HWH:HW],
                                    in_=x_d[row : row + 128, HWH:HW],
                                )
                                continue
                            nc.sync.dma_start(
                                out=ot[:, i, :],
                                in_=x_d[row + 128 * i : row + 128 * (i + 1), :],
                            )
                    else:
                        nc.sync.dma_start(
                            out=ot,
                            in_=x_d[row : row + 256, :].rearrange(
                                "(k p) f -> p k f", p=128
                            ),
                        )
                    ots.append(ot)
                all_ots.append(ots)
                all_pooled.append(
                    pooled_pool.tile([128, KC + 1, 2], F32, name=f"pooled{b}")
                )
                if b == 0:
                    # weights ride the sync ring AFTER sample 0's chunks:
                    # they aren't needed until matmul1 (~25us in), and
                    # triggering them first would delay the first pool ops
                    w1T = consts.tile([128, KC, HID], F32)
                    nc.sync.dma_start(
                        out=w1T, in_=w1_d.rearrange("p (k h) -> p k h", k=KC)
                    )
                    w2T = consts.tile([HID, KC, 128], F32)
                    nc.sync.dma_start(
                        out=w2T, in_=w2_d.rearrange("h (k p) -> h k p", k=KC)
                    )

            def act_sum(b, k, ph):
                with tc.tile_wait_until(ph):
                    nc.scalar.activation(
                        out=sink[:, 0:1].to_broadcast([128, HW]),
                        in_=all_ots[b][k // 2][:, k % 2, :],
                        func=AF.Copy,
                        scale=1.0 / HW,
                        accum_out=all_pooled[b][:, k, 0:1],
                    )

            def dve_sum(b, k, ph):
                # add-tree + CACHE_REDUCE(add): sum-pool on DVE for the
                # fill phase where DVE has slack and ACT is the backlog.
                # bf16 intermediates round ~2^-9 per level; the CR
                # accumulates in f32, well inside the 2e-2 error budget.
                with tc.tile_wait_until(ph):
                    src = all_ots[b][k // 2][:, k % 2, :]
                    s1 = tpool.tile([128, HWH], BF16, tag="s1", bufs=1,
                                    name=f"s1_{b}_{k}")
                    nc.vector.tensor_tensor(
                        out=s1, in0=src[0:128, 0:HWH], in1=src[0:128, HWH:HW],
                        op=ALU.add,
                    )
                    s2 = tpool.tile([128, HWQ], BF16, tag="s2", bufs=1,
                                    name=f"s2_{b}_{k}")
                    nc.vector.tensor_tensor(
                        out=s2, in0=s1[:, 0:HWQ], in1=s1[:, HWQ:HWH],
                        op=ALU.add,
                    )
                    nc.vector.tensor_scalar(
                        out=garbage[:, 0:HWQ],
                        in0=s2,
                        scalar1=1.0 / HW,
                        scalar2=None,
                        op0=ALU.mult,
                        op1=ALU.add,
                        accum_out=all_pooled[b][:, k, 0:1],
                    )

            HWE = HWQ // 2  # 392

            def max_tree(b, j, ph):
                with tc.tile_wait_until(ph):
                    ot = all_ots[b][j]
                    t1 = tpool.tile([128, 2, HWH], BF16, tag="t1", bufs=1,
                                    name=f"t1_{b}_{j}")
                    nc.vector.tensor_tensor(
                        out=t1, in0=ot[:, :, 0:HWH], in1=ot[:, :, HWH:HW],
                        op=ALU.max,
                    )
                    t2 = tpool.tile([128, 2, HWQ], BF16, tag="t2", bufs=1,
                                    name=f"t2_{b}_{j}")
                    nc.vector.tensor_tensor(
                        out=t2, in0=t1[:, :, 0:HWQ], in1=t1[:, :, HWQ:HWH],
                        op=ALU.max,
                    )
                    t3 = tpool.tile([128, 2, HWE], BF16, tag="t3", bufs=1,
                                    name=f"t3_{b}_{j}")
                    nc.vector.tensor_tensor(
                        out=t3, in0=t2[:, :, 0:HWE], in1=t2[:, :, HWE:HWQ],
                        op=ALU.max,
                    )
                    # one segmented reduce finishes BOTH chunks of the pair
                    nc.vector.tensor_reduce(
                        out=all_pooled[b][:, 2 * j : 2 * j + 2, 1],
                        in_=t3,
                        axis=mybir.AxisListType.X,
                        op=ALU.max,
                    )

            # ---- per-sample emission ----
            for b in range(B_LOC):
                ots = all_ots[b]
                pooled = all_pooled[b]
                last = b == B_LOC - 1

                # ACT sum-pools.  Window 0: chunks 0-4 on ACT in read-arrival
                # order, chunk 5 on DVE after the trees.  Steady state:
                # chunks 0-3 of sample b are fillers in window b-1's gate
                # chain; chunks 4,5 stay in window b.
                if b == 0:
                    # chunk 0 as two half-sums (halves land first); the
                    # second half accumulates into the spare pooled slot KC,
                    # folded back by an extra n=1 matmul1 term
                    with tc.tile_wait_until(0.005):
                        nc.scalar.activation(
                            out=sink[:, 0:1].to_broadcast([128, HWH]),
                            in_=ots[0][:, 0, 0:HWH],
                            func=AF.Copy, scale=1.0 / HW,
                            accum_out=pooled[:, 0, 0:1],
                        )
                    with tc.tile_wait_until(0.008):
                        nc.scalar.activation(
                            out=sink[:, 0:1].to_broadcast([128, HWH]),
                            in_=ots[0][:, 0, HWH:HW],
                            func=AF.Copy, scale=1.0 / HW,
                            accum_out=pooled[:, KC, 0:1],
                        )
                    for k in range(1, 5):
                        act_sum(0, k, 0.01 + 0.01 * k)
                else:
                    # NOTE: offloading sums to GpSimd tensor_tensor does NOT
                    # work: GpSimd shares an SBUF port with DVE, and running
                    # them concurrently was measured to slow DVE's pumped
                    # tensor ops 2.7x (MAX 1.7us -> 4.65us).
                    # (Moving c0 fillers to DVE add-trees was also tried:
                    # ERF_3 didn't move -- those windows aren't ACT-reach
                    # bound -- and it opened a new fabric gap.)
                    for k in range(4):
                        act_sum(b, k, (b - 1) + (0.90, 0.92, 0.94, 0.95)[k])
                    for k in range(4, KC):
                        act_sum(b, k, b + 0.10 + 0.02 * (k - 4))

                # DVE max-trees in read-arrival order.  ALL of sample b's
                # trees run BEFORE sample b-1's multiplies (trees-first):
                # sig_{b-1} is ready long before the trees finish, so DVE
                # never stalls, and the CACHE_REDUCEs are ready early for
                # matmul1/erf.
                if b == 0:
                    for j in range(KC // 2):
                        max_tree(0, j, 0.06 + 0.005 * j)
                    dve_sum(0, 5, 0.075)
                else:
                    for j in range(KC // 2):
                        max_tree(b, j, b + 0.10 + 0.01 * j)

                # matmul1: hT [48, 2] = sum_k w1T_k.T @ pooledT_k
                hps = psum_pool.tile([HID, 2], F32, tag="hps", name=f"hps{b}")
                for k in range(KC):
                    with tc.tile_wait_until(b + 0.30 + 0.01 * k):
                        nc.tensor.matmul(
                            hps,
                            w1T[:, k, :],
                            pooled[:, k, :],
                            start=(k == 0),
                            stop=(k == KC - 1),
                        )
                        if b == 0 and k == 0:
                            # fold the split first chunk's second half-sum
                            nc.tensor.matmul(
                                hps[:, 0:1],
                                w1T[:, 0, :],
                                pooled[:, KC, 0:1],
                                start=False,
                                stop=False,
                            )

                # gate chain: erf -> hh/hsum -> matmul2 -> sigmoid, ALL on
                # ACT.  Keeping hh/hsum off DVE matters: a DVE op waiting on
                # erf mid-queue serializes the next sample's max-trees behind
                # ACT's chain (measured 8us ACT idle before the last ERF).
                # hh_j = (e_j + 1) * u_j via the activation affine
                # (out = func(in*scale + bias)) with scale = bias = u_j.
                erf_ph = 0.91 if b == 0 else b + 0.945
                with tc.tile_wait_until(erf_ph - 0.001):
                    u_sb = small_pool.tile([HID, 2], F32, tag="u",
                                           name=f"u{b}")
                    nc.scalar.activation(out=u_sb, in_=hps, func=AF.Copy)
                with tc.tile_wait_until(erf_ph):
                    e_sb = small_pool.tile([HID, 2], F32, tag="e",
                                           name=f"e{b}")
                    nc.scalar.activation(
                        out=e_sb, in_=hps, func=AF.Erf, scale=0.7071067811865476
                    )
                with tc.tile_wait_until(erf_ph + 0.002):
                    hh = small_pool.tile([HID, 2], F32, tag="hh", name=f"hh{b}")
                    for jj in range(2):
                        nc.scalar.activation(
                            out=hh[:, jj : jj + 1], in_=e_sb[:, jj : jj + 1],
                            func=AF.Identity,
                            scale=u_sb[:, jj : jj + 1],
                            bias=u_sb[:, jj : jj + 1],
                        )
                    hsum = small_pool.tile([HID, 1], F32, tag="hsum",
                                           name=f"hsum{b}")
                    nc.scalar.activation(
                        out=hsum, in_=hh[:, 0:1], func=AF.Identity,
                        bias=hh[:, 1:2],
                    )
                mlp = psum_pool.tile([128, KC], F32, tag="mlp", name=f"mlp{b}")
                gate = small_pool.tile([128, KC], F32, tag="gate",
                                       name=f"gate{b}")
                sig_ph = 0.93 if b == 0 else b + 0.965
                if not last:
                    for k in range(KC):
                        with tc.tile_wait_until(erf_ph + 0.004 + 0.001 * k):
                            nc.tensor.matmul(
                                mlp[:, k : k + 1],
                                w2T[:, k, :],
                                hsum,
                                start=True,
                                stop=True,
                            )
                    with tc.tile_wait_until(sig_ph):
                        nc.scalar.activation(out=gate, in_=mlp, func=AF.Sigmoid)

                    # multiplies + writes: window b+1, after ALL of sample
                    # b+1's trees on DVE (trees-first ordering)
                    for j in range(KC // 2):
                        with tc.tile_wait_until(b + 1.16 + 0.01 * j):
                            ot = ots[j]
                            row = (b * KC + 2 * j) * 128
                            wt = bigpool.tile([128, 2, HW], BF16, tag="w",
                                              bufs=4, name=f"wt{b}_{j}")
                            for i in range(2):
                                k = 2 * j + i
                                nc.vector.tensor_scalar_mul(
                                    wt[:, i, :], ot[:, i, :], gate[:, k : k + 1]
                                )
                            out_ap = out_d[row : row + 256, :].rearrange(
                                "(k p) f -> p k f", p=128
                            )
                            nc.gpsimd.dma_start(out=out_ap, in_=wt)
                else:
                    # last sample: per-chunk matmul2 -> sigmoid -> mult ->
                    # write pipeline; writes spread across gpsimd/sync/
                    # scalar queues.  Write tiles reuse the steady-state
                    # "w" pair tag, half each.  (fp8 tail writes were tried:
                    # correct at 1.35e-2 rel err but NOT faster -- the DVE
                    # fp8-out mult loses its pump and SWDGE-cast serializes.)
                    wts = [
                        bigpool.tile([128, 2, HW], BF16, tag="w", bufs=4,
                                     name=f"wtl{j}")
                        for j in range(KC // 2)
                    ]
                    for k in range(KC):
                        with tc.tile_wait_until(b + 0.95 + 0.002 * k):
                            nc.tensor.matmul(
                                mlp[:, k : k + 1],
                                w2T[:, k, :],
                                hsum,
                                start=True,
                                stop=True,
                            )
                            nc.scalar.activation(
                                out=gate[:, k : k + 1], in_=mlp[:, k : k + 1],
                                func=AF.Sigmoid,
                            )
                        with tc.tile_wait_until(b + 0.96 + 0.002 * k):
                            ot = ots[k // 2]
                            row = (b * KC + k) * 128
                            wt = wts[k // 2][:, k % 2, :]
                            nc.vector.tensor_scalar_mul(
                                wt, ot[:, k % 2, :], gate[:, k : k + 1]
                            )
                            eng = (nc.gpsimd, nc.gpsimd, nc.sync, nc.sync,
                                   nc.scalar, nc.scalar)[k]
                            eng.dma_start(
                                out=out_d[row : row + 128, :], in_=wt
                            )
    nc.finalize()
    return nc


def kernel(x, w1, w2, _trace=False):
    if "nc" not in _cache:
        _cache["nc"] = _build_nc()
    nc = _cache["nc"]

    x = np.asarray(x).reshape(B, C, HW)
    w1t = np.ascontiguousarray(
        np.asarray(w1, np.float32).reshape(HID, KC, 128).transpose(2, 1, 0)
        .reshape(128, KC * HID)
    )
    w2t = np.ascontiguousarray(
        (0.5 * np.asarray(w2, np.float32)).reshape(KC, 128, HID)
        .transpose(2, 0, 1).reshape(HID, KC * 128)
    )
    in_maps = [
        {
            "x": np.ascontiguousarray(
                x[i * B_LOC : (i + 1) * B_LOC].reshape(B_LOC * C, HW)
            ).astype(ml_dtypes.bfloat16),
            "w1t": w1t,
            "w2t": w2t,
        }
        for i in range(NCORES)
    ]
    res = run_bass_kernel_spmd(nc, in_maps, core_ids=list(range(NCORES)),
                               trace=_trace)
    out = np.concatenate(
        [
            r["out"].astype(np.float32).reshape(B_LOC, C, 56, 56)
            for r in res.results
        ],
        axis=0,
    )
    if _trace:
        _cache["last_results"] = res
    return out
